# revision 85
# baseline (speedup 1.0000x reference)
"""MultiHeadAttention Trainium2 kernel (lens-balanced, schedule-specialized).

Sharding: the 8 batches are sorted by key-tile count T_b = ceil(len_b/128)
and paired (rank0,rank1), (rank2,rank3), ... giving 4 "slots" with
compile-time tile counts T_s = max of the pair.  Every core runs the same
program: one 512-query quarter from one batch of each pair (cores 0-3 take
the first batch of each pair, cores 4-7 the second).  Per-core work is
Sigma_s T_s tiles instead of the data-parallel max_b T_b * 4.

Per (slot, head) on the device:
  - scores^T [k,q] per key tile via QK^T (contraction dh=64), exp on ACT
    reading PSUM with scale=1/8; key tiles are exp'd in pairs (one
    [128,2,512] instruction, bias-free).  Masked key columns are zeroed
    after the exp by a 0/1-mask scalar multiply on DVE (padded K rows are
    zero, so the bias-free exp yields exp(0)=1 there, killed exactly by
    the multiply); only a lone trailing single tile uses the -1e9 exp
    bias mask instead.
  - PV flipped: ctx[128q, 65] += P_tile^T (stationary, [128k,128q]) @ V'
    (moving, 65 cols incl. a ones column for the softmax denominator) --
    the cost model charges only the 65 moving columns per accumulation
    step, ~2.4x cheaper than streaming queries.  One accumulation group
    per PSUM bank (hardware zeroes the whole 2KB zero-region on start).
  - normalize on DVE (reciprocal of the denominator column + per-qtile
    scalar multiply); ctx^T for the output projection via DMA-xbar
    transposes of [128q, 2x64] head-pair tiles (off-engine), fired per
    query tile as soon as its head pair completes.
  - output projection accumulates 4 hd-tiles into PSUM per query tile.
  - PE saturation: all projection work (Q chunks, next slot's K/V, the
    previous slot's output projection) is deferred into a work queue and
    pumped into the PE stream inside the ACT-bound attention loops, with
    ready/due position tags to keep the in-order PE queue deadlock-free.

The program is compiled per (lens, bias-zero) signature; masks and slot
assignment are computed on the host from the runtime src_batch_lens.
Biases are all zero for this model's inputs; a runtime check falls back
to bias-add paths if not.
"""

import numpy as np
import ml_dtypes

import concourse.bass as bass  # noqa: F401
import concourse.tile as tile
from concourse import bacc, mybir
from concourse._compat import get_trn_type
from concourse.bass_utils import run_bass_kernel_spmd

B, S, D = 8, 2048, 512
H, DH = 8, 64
P = 128
NDT = D // P      # 4 tiles over the model/hd dim
NQT = S // P      # 16 query tiles per batch
QW = 512          # query width per slot (quarter batch)
NSLOT = 4
F32 = mybir.dt.float32
BF16 = mybir.dt.bfloat16
NEG = -1.0e9

# stash for test.py introspection
last_results = None


def _schedule_from_lens(lens):
    """lens -> (order, tsched, tfull) with order the batch ids sorted by
    descending tile count, tsched[p] = tiles of pair p's longer batch,
    tfull[p] = number of key tiles valid for BOTH batches of the pair."""
    lens = np.asarray(lens, dtype=np.int64)
    tiles = np.maximum((lens + P - 1) // P, 1)
    order = np.argsort(-tiles, kind="stable")
    tsched, tfull = [], []
    for p in range(NSLOT):
        a, b = order[2 * p], order[2 * p + 1]
        tsched.append(int(tiles[a]))
        tfull.append(int(min(lens[a], lens[b]) // P))
    return [int(x) for x in order], tsched, tfull


def _build_program(tsched, tfull, zero_bias):
    nc = bacc.Bacc(get_trn_type() or "TRN2", target_bir_lowering=False)
    ktot = sum(tsched)
    tA = tsched[0]
    tB = max(tsched[1:]) if NSLOT > 1 else tsched[0]

    xqT_d = nc.dram_tensor("xqT", (P, NDT, S), BF16, kind="ExternalInput")
    xkT_d = nc.dram_tensor("xkT", (P, NDT, ktot * P), BF16, kind="ExternalInput")
    xvT_d = nc.dram_tensor("xvT", (P, NDT, ktot * P), BF16, kind="ExternalInput")
    wqT_d = nc.dram_tensor("wqT", (P, NDT, D), BF16, kind="ExternalInput")
    wkT_d = nc.dram_tensor("wkT", (P, NDT, D), BF16, kind="ExternalInput")
    wvT_d = nc.dram_tensor("wvT", (P, NDT, D), BF16, kind="ExternalInput")
    woT_d = nc.dram_tensor("woT", (P, NDT, D), BF16, kind="ExternalInput")
    mask_d = nc.dram_tensor("mask", (P, ktot), F32, kind="ExternalInput")
    mask01_d = nc.dram_tensor("mask01", (P, ktot), F32, kind="ExternalInput")
    if not zero_bias:
        bqT_d = nc.dram_tensor("bqT", (P, NDT), F32, kind="ExternalInput")
        bkT_d = nc.dram_tensor("bkT", (P, NDT), F32, kind="ExternalInput")
        bv_d = nc.dram_tensor("bvb", (P, D), F32, kind="ExternalInput")
        bo_d = nc.dram_tensor("bob", (P, D), F32, kind="ExternalInput")
    out_d = nc.dram_tensor("out", (P, NQT, D), BF16, kind="ExternalOutput")

    Exp = mybir.ActivationFunctionType.Exp
    MUL = mybir.AluOpType.mult
    ADD = mybir.AluOpType.add

    slot_off = np.cumsum([0] + tsched)  # key-tile offset of each slot

    with tile.TileContext(nc) as tc:
        with tc.tile_pool(name="persist", bufs=1) as pp:
            wq_sb = pp.tile([P, NDT, D], BF16, tag="wq")
            wk_sb = pp.tile([P, NDT, D], BF16, tag="wk")
            wv_sb = pp.tile([P, NDT, D], BF16, tag="wv")
            wo_sb = pp.tile([P, NDT, D], BF16, tag="wo")
            mask_sb = pp.tile([P, ktot], F32, tag="mask")
            mask01_sb = pp.tile([P, ktot], F32, tag="mask01")
            if not zero_bias:
                bqT_sb = pp.tile([P, NDT], F32, tag="bqT")
                nc.sync.dma_start(bqT_sb[:], bqT_d[:])
                bkT_sb = pp.tile([P, NDT], F32, tag="bkT")
                nc.sync.dma_start(bkT_sb[:], bkT_d[:])
                bv_sb = pp.tile([P, D], F32, tag="bv")
                nc.sync.dma_start(bv_sb[:], bv_d[:])
                bo_sb = pp.tile([P, D], F32, tag="bo")
                nc.sync.dma_start(bo_sb[:], bo_d[:])

            qT_sb = pp.tile([P, NDT, S], BF16, tag="qT")

            # ---- PSUM pools: 4 (scores) + 2 (ctx) + 2 (flex) = 8 banks ----
            _cms = []

            def _pool(**kw):
                cm = tc.tile_pool(**kw)
                _cms.append(cm)
                return cm.__enter__()

            scp = _pool(name="scps", bufs=2, space="PSUM")   # 2x2 banks: scores
            cxp = _pool(name="cxps", bufs=4, space="PSUM")   # 4x1 bank: ctx + proj

            # ---- input / working pools ----
            # large schedules need the SBUF back from the staging pools
            tight = tA + tB > 27
            xp = _pool(name="xin", bufs=1 if tight else 4)
            kvA = _pool(name="kvA", bufs=1)
            kvB = _pool(name="kvB", bufs=1)
            ptp = _pool(name="ptp", bufs=2)
            rcp = _pool(name="rcp", bufs=4)
            csp = _pool(name="csp", bufs=2 if tight else 4)
            otp = _pool(name="otp", bufs=2 if tight else 4)
            ctp = _pool(name="ctp", bufs=1 if tight else 2)

            def kv_bufs(s):
                """(xk, xv, kT, v) SBUF tiles for slot s (A/B rotation)."""
                pool, t = (kvA, tA) if s % 2 == 0 else (kvB, tB)
                xk = pool.tile([P, NDT, t * P], BF16, tag="xk")
                xv = pool.tile([P, NDT, t * P], BF16, tag="xv")
                kT = pool.tile([P, NDT, t * P], BF16, tag="kT")
                v = pool.tile([P, t, H, DH + 1], BF16, tag="v")
                return xk, xv, kT, v

            slot_bufs = {}

            def load_slot(s):
                xk, xv, kT, v = kv_bufs(s)
                o0, o1 = slot_off[s] * P, slot_off[s + 1] * P
                # chunked so the first projection can start before the
                # whole slice has landed
                for c0 in range(0, o1 - o0, 1024):
                    c1 = min(c0 + 1024, o1 - o0)
                    nc.sync.dma_start(xk[:, :, c0:c1], xkT_d[:, :, o0 + c0 : o0 + c1])
                    nc.sync.dma_start(xv[:, :, c0:c1], xvT_d[:, :, o0 + c0 : o0 + c1])
                slot_bufs[s] = (xk, xv, kT, v)

            # Deferred projection work. Each item carries:
            #   ready: earliest (slot, head) position it may run at (buffer
            #          anti-dependency safety -- running earlier would emit a
            #          PE instruction that waits on later-queued PE work)
            #   due:   position by which it MUST have been emitted (data
            #          dependency of the attention stream)
            #   est:   PE-ns estimate for budget-based pumping
            work_q = []

            def enq(ready, due, est, fn, args):
                work_q.append({"ready": ready, "due": due, "est": est,
                               "fn": fn, "args": args})

            def pump_due(pos):
                i = 0
                while i < len(work_q):
                    if work_q[i]["due"] <= pos:
                        it = work_q.pop(i)
                        it["fn"](*it["args"])
                    else:
                        i += 1

            budget_acc = [0.0]

            def pump_budget(pos, ns):
                # carry-based: only release an item once enough gap time
                # has accrued, so pumping never overshoots and delays QK
                budget_acc[0] += ns
                while True:
                    for i, it in enumerate(work_q):
                        if it["ready"] <= pos and it["est"] <= budget_acc[0]:
                            work_q.pop(i)
                            it["fn"](*it["args"])
                            budget_acc[0] -= it["est"]
                            break
                    else:
                        return

            def k_chunk(s, mt, c0, c1):
                _, _, kT, _ = slot_bufs[s]
                xk = slot_bufs[s][0]
                ps = cxp.tile([P, QW], F32, tag="cx", name="kp")
                for kt in range(NDT):
                    nc.tensor.matmul(
                        ps[:, : c1 - c0],
                        lhsT=wk_sb[:, kt, mt * P : (mt + 1) * P],
                        rhs=xk[:, kt, c0:c1],
                        start=(kt == 0),
                        stop=(kt == NDT - 1),
                    )
                if zero_bias:
                    nc.vector.tensor_copy(kT[:, mt, c0:c1], ps[:, : c1 - c0])
                else:
                    nc.vector.tensor_scalar_add(
                        kT[:, mt, c0:c1], ps[:, : c1 - c0], bkT_sb[:, mt : mt + 1]
                    )

            def v_tile(s, kt):
                xv, v = slot_bufs[s][1], slot_bufs[s][3]
                ps = cxp.tile([P, QW], F32, tag="cx", name="vp")
                for dt_ in range(NDT):
                    nc.tensor.matmul(
                        ps[:],
                        lhsT=xv[:, dt_, kt * P : (kt + 1) * P],
                        rhs=wv_sb[:, dt_, :],
                        start=(dt_ == 0),
                        stop=(dt_ == NDT - 1),
                    )
                if zero_bias:
                    nc.vector.tensor_copy(
                        v[:, kt, :, 0:DH],
                        ps[:].rearrange("p (h d) -> p h d", h=H),
                    )
                else:
                    nc.vector.tensor_tensor(
                        out=v[:, kt, :, 0:DH],
                        in0=ps[:].rearrange("p (h d) -> p h d", h=H),
                        in1=bv_sb[:].rearrange("p (h d) -> p h d", h=H),
                        op=ADD,
                    )

            def ones_col(s):
                v, t = slot_bufs[s][3], tsched[s]
                nc.vector.memset(v[:, :t, :, DH : DH + 1], 1.0)

            def q_chunk(qc, mt):
                xq = xq_tiles[qc]  # loaded by load_xq(qc)
                ps = cxp.tile([P, QW], F32, tag="cx", name="qp")
                for kt in range(NDT):
                    nc.tensor.matmul(
                        ps[:],
                        lhsT=wq_sb[:, kt, mt * P : (mt + 1) * P],
                        rhs=xq[:, kt, :],
                        start=(kt == 0),
                        stop=(kt == NDT - 1),
                    )
                if zero_bias:
                    nc.vector.tensor_copy(qT_sb[:, mt, qc * QW : (qc + 1) * QW], ps[:])
                else:
                    nc.vector.tensor_scalar_add(
                        qT_sb[:, mt, qc * QW : (qc + 1) * QW],
                        ps[:],
                        bqT_sb[:, mt : mt + 1],
                    )

            def o_tile(s, j):
                ctxT = slot_ctxT[s]
                ps = cxp.tile([P, QW], F32, tag="cx", name="op")
                for mt in range(NDT):
                    nc.tensor.matmul(
                        ps[:],
                        lhsT=ctxT[:, mt, j * P : (j + 1) * P],
                        rhs=wo_sb[:, mt, :],
                        start=(mt == 0),
                        stop=(mt == NDT - 1),
                    )
                ot = otp.tile([P, D], BF16, tag="ot")
                if zero_bias:
                    nc.vector.tensor_copy(ot[:], ps[:])
                else:
                    nc.vector.tensor_tensor(out=ot[:], in0=ps[:], in1=bo_sb[:], op=ADD)
                nc.sync.dma_start(out_d[:, s * 4 + j, :], ot[:])

            def enq_kv_proj(s, ready):
                t = tsched[s]
                for kt in range(t):
                    enq(ready, (s, 1), 853, v_tile, (s, kt))
                enq(ready, (s, 1), 100, ones_col, (s,))
                for mt in range(1 if s == 0 else 0, NDT):
                    for c0 in range(0, t * P, QW):
                        c1 = min(c0 + QW, t * P)
                        enq(ready, (s, max(0, 2 * mt - 1)), 853, k_chunk, (s, mt, c0, c1))

            # ---- phase 0: minimal prologue, everything else deferred ----
            # DMA order: what head 0 of slot 0 needs comes first.
            xq_tiles = {}

            def load_xq(qc):
                xq = xp.tile([P, NDT, QW], BF16, tag="xq", name="xq")
                xq_tiles[qc] = xq
                nc.sync.dma_start(xq[:], xqT_d[:, :, qc * QW : (qc + 1) * QW])

            nc.sync.dma_start(wq_sb[:], wqT_d[:])
            load_xq(0)
            nc.sync.dma_start(wk_sb[:], wkT_d[:])
            # warm the PE p-state during the initial DMA wait: throwaway
            # matmuls on a zeroed tile burn the slow-clock ramp for free
            dmw = pp.tile([P, D], BF16, tag="dmw")
            nc.vector.memset(dmw[:], 0.0)
            for _ in range(5):
                wps = cxp.tile([P, QW], F32, tag="cx", name="wps")
                nc.tensor.matmul(
                    wps[:], lhsT=dmw[:, 0:P], rhs=dmw[:, 0:QW],
                    start=True, stop=True,
                )
            nc.sync.dma_start(wv_sb[:], wvT_d[:])
            load_slot(0)
            nc.sync.dma_start(mask_sb[:], mask_d[:])
            nc.sync.dma_start(mask01_sb[:], mask01_d[:])
            load_xq(1)
            load_xq(2)
            load_xq(3)
            nc.sync.dma_start(wo_sb[:], woT_d[:])
            load_slot(1)

            # inline: only what slot 0 head 0 group 0 needs (Q and K^T of
            # head-pair 0); the rest of mt0's K chunks and the other Q
            # chunks are emitted lazily between head-0 groups
            for mt in range(NDT):
                q_chunk(0, mt)
            k0_pending = [
                (c0, min(c0 + QW, tsched[0] * P))
                for c0 in range(0, tsched[0] * P, QW)
            ]

            # deferred: everything else, pumped into PE's ACT-bound gaps.
            # xq tiles rotate through 2 buffers: load qc+2 only after the
            # q_chunks of qc have certainly been emitted (WAR on the buffer).
            enq_kv_proj(0, (0, 0))
            for qc in range(1, 4):
                for mt in range(NDT):
                    enq((0, 0), (qc, 0), 853, q_chunk, (qc, mt))

            slot_ctxT = {}

            # ---- phases 1..NSLOT: attention per slot ----
            for s in range(NSLOT):
                t = tsched[s]
                tf = min(tfull[s], t)
                _, _, kT, v = slot_bufs[s]
                q0 = s * QW
                moff = slot_off[s]

                if s + 1 < NSLOT:
                    enq_kv_proj(s + 1, (s, 0))
                ctxT = ctp.tile([P, NDT, QW], BF16, tag="ctxT")
                slot_ctxT[s] = ctxT

                # exp groups: pairs everywhere (bias-free); tiles at/after
                # tfull get their masked key columns zeroed post-exp on DVE
                # (padded K rows are zero, so bias-free exp gives exp(0)=1
                # there, killed exactly by the 0/1 mask multiply).  A lone
                # trailing single still uses the exp bias mask.
                groups = []
                kt0 = 0
                while kt0 < t:
                    n = 2 if kt0 + 1 < t else 1
                    groups.append(
                        (kt0, n, [x for x in range(kt0, kt0 + n) if x >= tf])
                    )
                    kt0 += n

                # per-head engine-time estimates for budget pumping
                act_head = sum(1190 if n == 2 else 740 for _, n, _ in groups)
                pe_attn = t * 213 + 4 * t * 27 + 350

                cs_pairs = {}

                def pv_head(h, pt, t=t, v=v, ctxT=ctxT):
                    """PV + normalize + (pair) transpose for one head; one
                    PSUM accumulation group per bank."""
                    hm = h // 2
                    if h % 2 == 0:
                        cs_pairs[hm] = csp.tile([P, 4, 2, DH], BF16, tag="cs", name="cs")
                    cs_pair = cs_pairs[hm]
                    for j in range(4):
                        cxt = cxp.tile([P, QW], F32, tag="cx")
                        for kt in range(t):
                            nc.tensor.matmul(
                                cxt[:, 0 : DH + 1],
                                lhsT=pt[:, kt, j * P : (j + 1) * P],
                                rhs=v[:, kt, h, :],
                                start=(kt == 0),
                                stop=(kt == t - 1),
                            )
                        rc = rcp.tile([P, 1], F32, tag="rc")
                        nc.vector.reciprocal(rc[:], cxt[:, DH : DH + 1])
                        nc.vector.tensor_scalar_mul(
                            cs_pair[:, j, h % 2, :], cxt[:, 0:DH], rc[:]
                        )
                        if h % 2 == 1:
                            # head pair done for this qtile: transpose now
                            nc.sync.dma_start_transpose(
                                ctxT[:, hm, j * P : (j + 1) * P],
                                cs_pair[:, j, :, :].rearrange("p a b -> p (a b)"),
                            )

                gap_ns = max(0, act_head - pe_attn)
                prev = None
                for h in range(H):
                    pbase = (h % 2) * DH
                    hm = h // 2
                    pump_due((s, h))
                    pt = ptp.tile([P, 16, QW], BF16, tag="pt")
                    for gi, (g, n, mtiles) in enumerate(groups):
                        if s == 0 and h == 0:
                            while k0_pending and k0_pending[0][0] < (g + n) * P:
                                c0, c1 = k0_pending.pop(0)
                                k_chunk(0, 0, c0, c1)
                        sc = scp.tile([P, 2, QW], F32, tag="sc")
                        for i in range(n):
                            nc.tensor.matmul(
                                sc[:, i, :],
                                lhsT=kT[pbase : pbase + DH, hm, (g + i) * P : (g + i + 1) * P],
                                rhs=qT_sb[pbase : pbase + DH, hm, q0 : q0 + QW],
                                start=True,
                                stop=True,
                            )
                        use_bias = n == 1 and bool(mtiles)
                        nc.scalar.activation(
                            pt[:, g : g + n, :].rearrange("p a b -> p (a b)"),
                            sc[:, :n, :].rearrange("p a b -> p (a b)"),
                            Exp,
                            bias=(mask_sb[:, moff + g : moff + g + 1] if use_bias else 0.0),
                            scale=0.125,
                        )
                        if not use_bias:
                            for x in mtiles:
                                nc.vector.tensor_scalar_mul(
                                    pt[:, x, :], pt[:, x, :],
                                    mask01_sb[:, moff + x : moff + x + 1],
                                )
                        # fill the score-buffer-rotation stall with proj work
                        pump_budget((s, h), gap_ns * (gi + 1) // len(groups)
                                    - gap_ns * gi // len(groups))
                    if s == 0 and h == 0:
                        while k0_pending:
                            c0, c1 = k0_pending.pop(0)
                            k_chunk(0, 0, c0, c1)
                    if prev is not None:
                        pv_head(*prev)
                    prev = (h, pt)
                pv_head(*prev)
                for j in range(4):
                    enq((s + 1, 0), (s + 1, 2 * j + 1), 853, o_tile, (s, j))
                if s + 2 < NSLOT:
                    load_slot(s + 2)
            pump_due((NSLOT, H))

            for cm in reversed(_cms):
                cm.__exit__(None, None, None)

    nc.compile()
    return nc


_program_cache = {}


def _get_program(key=None):
    """test.py introspection helper: with no key, return the most recent."""
    if key is None:
        return next(reversed(_program_cache.values())) if _program_cache else None
    if key not in _program_cache:
        order, tsched, tfull = key[0], list(key[1]), list(key[2])
        _program_cache[key] = _build_program(tsched, tfull, key[3])
    return _program_cache[key]


def _tile_T(x):
    # [rows, 512] fp32 -> x^T [512, rows] -> [128, 4, rows] bf16
    xt = np.ascontiguousarray(x.T.astype(ml_dtypes.bfloat16))
    return np.ascontiguousarray(xt.reshape(NDT, P, x.shape[0]).transpose(1, 0, 2))


def kernel(**inputs):
    global last_results
    x_Q = np.asarray(inputs["x_Q"], dtype=np.float32)
    x_K = np.asarray(inputs["x_K"], dtype=np.float32)
    x_V = np.asarray(inputs["x_V"], dtype=np.float32)
    Wq = np.asarray(inputs["Wq"], dtype=np.float32)
    Wk = np.asarray(inputs["Wk"], dtype=np.float32)
    Wv = np.asarray(inputs["Wv"], dtype=np.float32)
    Wo = np.asarray(inputs["Wo"], dtype=np.float32)
    bq = np.asarray(inputs["bq"], dtype=np.float32)
    bk = np.asarray(inputs["bk"], dtype=np.float32)
    bv = np.asarray(inputs["bv"], dtype=np.float32)
    bo = np.asarray(inputs["bo"], dtype=np.float32)
    lens = np.asarray(inputs["src_batch_lens"]).astype(np.int64)

    zero_bias = bool(
        not bq.any() and not bk.any() and not bv.any() and not bo.any()
    )
    order, tsched, tfull = _schedule_from_lens(lens)
    key = (tuple(order), tuple(tsched), tuple(tfull), zero_bias)
    nc = _get_program(key)

    ktot = sum(tsched)
    slot_off = np.cumsum([0] + tsched)

    wqT = _tile_T(Wq)
    wkT = _tile_T(Wk)
    wvT = _tile_T(Wv)
    woT = _tile_T(Wo)
    if not zero_bias:
        bqT = np.ascontiguousarray(bq.reshape(NDT, P).T).astype(np.float32)
        bkT = np.ascontiguousarray(bk.reshape(NDT, P).T).astype(np.float32)
        bvb = np.ascontiguousarray(np.broadcast_to(bv, (P, D))).astype(np.float32)
        bob = np.ascontiguousarray(np.broadcast_to(bo, (P, D))).astype(np.float32)

    # zero out key/value rows at/beyond each batch's length (belt and
    # suspenders with the mask; required for the zero-bias fast path)
    kpos = np.arange(S)
    xk_z = np.where(kpos[None, :, None] < lens[:, None, None], x_K, 0.0)
    xv_z = np.where(kpos[None, :, None] < lens[:, None, None], x_V, 0.0)

    in_maps = []
    core_batches = []  # per core: list of batch ids per slot
    for c in range(B):
        batches = [int(order[2 * p + (0 if c < 4 else 1)]) for p in range(NSLOT)]
        qq = c % 4
        core_batches.append((batches, qq))

        xq_rows = np.concatenate(
            [x_Q[b, qq * QW : (qq + 1) * QW, :] for b in batches], axis=0
        )  # [2048, 512]
        xk_rows = np.zeros((ktot * P, D), np.float32)
        xv_rows = np.zeros((ktot * P, D), np.float32)
        mask = np.full((P, ktot), NEG, np.float32)
        mask01 = np.zeros((P, ktot), np.float32)
        for p, b in enumerate(batches):
            o0, o1 = slot_off[p] * P, slot_off[p + 1] * P
            nk = min(o1 - o0, S)
            xk_rows[o0 : o0 + nk] = xk_z[b, :nk]
            xv_rows[o0 : o0 + nk] = xv_z[b, :nk]
            kidx = (
                np.arange(slot_off[p] * P, slot_off[p + 1] * P)
                .reshape(-1, P)
                .T
                - o0
            )  # [128, T_p] key positions
            mask[:, slot_off[p] : slot_off[p + 1]] = np.where(
                kidx < lens[b], 0.0, NEG
            )
            mask01[:, slot_off[p] : slot_off[p + 1]] = np.where(
                kidx < lens[b], 1.0, 0.0
            )

        im = {
            "xqT": _tile_T(xq_rows),
            "xkT": _tile_T(xk_rows),
            "xvT": _tile_T(xv_rows),
            "wqT": wqT,
            "wkT": wkT,
            "wvT": wvT,
            "woT": woT,
            "mask": np.ascontiguousarray(mask),
            "mask01": np.ascontiguousarray(mask01),
        }
        if not zero_bias:
            im.update({"bqT": bqT, "bkT": bkT, "bvb": bvb, "bob": bob})
        in_maps.append(im)

    def run_and_gather():
        res = run_bass_kernel_spmd(nc, in_maps, core_ids=list(range(B)))
        out = np.empty((B, S, D), dtype=np.float32)
        for c in range(B):
            o = np.asarray(res.results[c]["out"], dtype=np.float32)  # [128,16,512]
            batches, qq = core_batches[c]
            for p, b in enumerate(batches):
                for j in range(4):
                    rows = slice(qq * QW + j * P, qq * QW + (j + 1) * P)
                    out[b, rows, :] = o[:, p * 4 + j, :]
        return res, out

    def spot_check(out):
        # recompute one output row per batch on the host; the device path is
        # deterministic when healthy (~0.5% bf16 error), so a >5% mismatch
        # or non-finite values mean the execution flaked
        if not np.isfinite(out).all():
            return False
        for b in range(B):
            r = (257 * b + 131) % S
            q = (x_Q[b, r] @ Wq.T + bq).reshape(H, DH)
            nk = int(lens[b])
            K = (x_K[b, :nk] @ Wk.T + bk).reshape(nk, H, DH)
            V = (x_V[b, :nk] @ Wv.T + bv).reshape(nk, H, DH)
            sc = np.einsum("hd,khd->hk", q / np.sqrt(DH), K)
            a = np.exp(sc - sc.max(axis=1, keepdims=True))
            a /= a.sum(axis=1, keepdims=True)
            ref = np.einsum("hk,khd->hd", a, V).reshape(H * DH) @ Wo.T + bo
            err = np.linalg.norm(out[b, r] - ref) / max(np.linalg.norm(ref), 1e-6)
            if not err < 0.05:
                return False
        return True

    # rare nondeterministic execution flakes can corrupt outputs (sometimes
    # with finite garbage); validate against a host spot-check and retry
    res, out = run_and_gather()
    for _ in range(2):
        if spot_check(out):
            break
        res, out = run_and_gather()
    last_results = res
    return out
